# revision 20
# baseline (speedup 1.0000x reference)
"""MinGRU cell kernel for Trainium2 (8 NeuronCores, data-parallel over batch).

Computes, for x:[B,T,D], motion_mag:[B,T]:
    tau = 1 + softplus(alpha) * sigmoid(mw*mm + mb)        (per b,t)
    z   = sigmoid((x @ Wz^T + bz) / tau)                   (B,T,H)
    ht  = x @ Wh^T + bh                                    (B,T,H)
    h_t = (1-z_t)*h_{t-1} + z_t*ht_t   (scan over t, h_0=0)

Strategy:
  - Shard B=32 across 8 cores (4 per core). Weights replicated.
  - On-chip layout: h on partitions, t on the free dim, so the recurrence is
    a HW tensor_tensor_scan per [128h, 512t] tile, carried across t-tiles via
    initial=prev[:, -1:].
  - Projections: lhsT = W^T chunks (stationary), rhs = x^T chunks (moving),
    float32r (full PE rate, near-fp32 accuracy, fp32 PSUM accumulation).
  - tau: 1/tau computed on host, DMA-broadcast across partitions per block;
    folded in via one fused scalar_tensor_tensor: u = (zpre + bz) * invtau.
  - z = sigmoid(u), a = sigmoid(-u) = 1-z on ACT; b = (hpre + bh) * z on DVE.
  - Host pre-transposes x to [d, b*t] per core and un-transposes the output.
"""

import sys

import numpy as np

if "/opt/trn_rl_repo" not in sys.path:
    sys.path.insert(0, "/opt/trn_rl_repo")

B, T, D, H = 32, 2048, 512, 512
NCORES = 8
BL = B // NCORES            # batch per core = 4
TBLK = 1024                 # t-columns per block (2 psum banks)
MMN = 512                   # matmul free-dim (1 psum bank)
NTB = T // TBLK             # 2 t-blocks per sample
DC = D // 128               # 4 contraction chunks
HC = H // 128               # 4 h partition chunks
BT = BL * T                 # 8192 columns per core

_CACHE = {}


def _build_nc():
    import concourse.bass as bass
    import concourse.bacc as bacc
    import concourse.mybir as mybir
    import concourse.tile as tile
    from contextlib import ExitStack

    f32 = mybir.dt.float32
    f32r = mybir.dt.float32r
    AF = mybir.ActivationFunctionType
    OP = mybir.AluOpType

    nc = bacc.Bacc("TRN2", target_bir_lowering=False, debug=False)

    xt_ext = nc.declare_dram_parameter("xt", [DC, 128, BT], f32r, isOutput=False)
    wzt_ext = nc.declare_dram_parameter("wzt", [DC, 128, H], f32r, isOutput=False)
    wht_ext = nc.declare_dram_parameter("wht", [DC, 128, H], f32r, isOutput=False)
    bz_ext = nc.declare_dram_parameter("bz", [HC, 128, 1], f32, isOutput=False)
    bh_ext = nc.declare_dram_parameter("bh", [HC, 128, 1], f32, isOutput=False)
    itau_ext = nc.declare_dram_parameter("invtau", [BL, 1, T], f32, isOutput=False)
    out_ext = nc.declare_dram_parameter("out", [BL, HC, 128, T], f32, isOutput=True)

    with tile.TileContext(nc) as tc, ExitStack() as ctx:
        singles = ctx.enter_context(tc.tile_pool(name="singles", bufs=1))
        x_pool = ctx.enter_context(tc.tile_pool(name="x", bufs=2))
        j_pool = ctx.enter_context(tc.tile_pool(name="j", bufs=3))
        psum = ctx.enter_context(tc.tile_pool(name="psum", bufs=2, space="PSUM"))
        work = ctx.enter_context(tc.tile_pool(name="work", bufs=3))
        ab_pool = ctx.enter_context(tc.tile_pool(name="ab", bufs=3))
        h_pool = ctx.enter_context(tc.tile_pool(name="h", bufs=6))

        # First block's x tiles, issued before everything else so the PE can
        # start as soon as the (smaller) weight DMAs land.
        xs0 = []
        for dc in range(DC):
            xt = x_pool.tile([128, TBLK], f32r, tag=f"x{dc}", name=f"x0_{dc}")
            nc.sync.dma_start(out=xt[:], in_=xt_ext[dc, :, 0:TBLK])
            xs0.append(xt)

        # Stationary weights: one fused DMA each (free dim = (dc, h)).
        wz_all = singles.tile([128, DC * H], f32r, tag="wza")
        wh_all = singles.tile([128, DC * H], f32r, tag="wha")
        for w_all, w_ext in ((wz_all, wzt_ext), (wh_all, wht_ext)):
            base = w_ext[0]
            w_in = bass.AP(
                tensor=base.tensor, offset=base.offset,
                ap=[[H, 128], [128 * H, DC], [1, H]],
            )
            nc.sync.dma_start(out=w_all[:], in_=w_in)
        wz_sb = [wz_all[:, dc * H:(dc + 1) * H] for dc in range(DC)]
        wh_sb = [wh_all[:, dc * H:(dc + 1) * H] for dc in range(DC)]
        bz_col, bh_col = [], []
        for hc in range(HC):
            bzc = singles.tile([128, 1], f32, tag=f"bz{hc}")
            nc.gpsimd.dma_start(out=bzc[:], in_=bz_ext[hc])
            bz_col.append(bzc)
            bhc = singles.tile([128, 1], f32, tag=f"bh{hc}")
            nc.gpsimd.dma_start(out=bhc[:], in_=bh_ext[hc])
            bh_col.append(bhc)

        h_prev = [[None] * HC for _ in range(BL)]

        for b in range(BL):
            for tb in range(NTB):
                bt0 = b * T + tb * TBLK
                ts = slice(tb * TBLK, (tb + 1) * TBLK)
                if b == 0 and tb == 0:
                    xs = xs0
                else:
                    xs = []
                    for dc in range(DC):
                        xt = x_pool.tile([128, TBLK], f32r, tag=f"x{dc}")
                        nc.sync.dma_start(
                            out=xt[:], in_=xt_ext[dc, :, bt0:bt0 + TBLK]
                        )
                        xs.append(xt)
                # Broadcast 1/tau row across all 128 partitions.
                jt = j_pool.tile([128, TBLK], f32, tag="J")
                iv = itau_ext[b, 0, tb * TBLK:(tb + 1) * TBLK]
                iv_b = bass.AP(
                    tensor=iv.tensor, offset=iv.offset, ap=[[0, 128]] + list(iv.ap)
                )
                nc.gpsimd.dma_start(out=jt[:], in_=iv_b)

                for hc in range(HC):
                    hsl = slice(hc * 128, (hc + 1) * 128)
                    zq = psum.tile([128, TBLK], f32, tag="zq")
                    hq = psum.tile([128, TBLK], f32, tag="hq")
                    for half in range(TBLK // MMN):
                        csl = slice(half * MMN, (half + 1) * MMN)
                        for dc in range(DC):
                            nc.tensor.matmul(
                                zq[:, csl],
                                lhsT=wz_sb[dc][:, hsl],
                                rhs=xs[dc][:, csl],
                                start=(dc == 0),
                                stop=(dc == DC - 1),
                            )
                        for dc in range(DC):
                            nc.tensor.matmul(
                                hq[:, csl],
                                lhsT=wh_sb[dc][:, hsl],
                                rhs=xs[dc][:, csl],
                                start=(dc == 0),
                                stop=(dc == DC - 1),
                            )

                    # u = (zpre + bz) * invtau
                    u = work.tile([128, TBLK], f32, tag="u")
                    nc.vector.scalar_tensor_tensor(
                        u[:], zq[:], bz_col[hc][:], jt[:, :],
                        op0=OP.add, op1=OP.mult,
                    )
                    z = work.tile([128, TBLK], f32, tag="z")
                    nc.scalar.activation(z[:], u[:], AF.Sigmoid)
                    a = ab_pool.tile([128, TBLK], f32, tag="a")
                    nc.scalar.activation(a[:], u[:], AF.Sigmoid, scale=-1.0)
                    # b = (hpre + bh) * z
                    bb = ab_pool.tile([128, TBLK], f32, tag="b")
                    nc.vector.scalar_tensor_tensor(
                        bb[:], hq[:], bh_col[hc][:], z[:],
                        op0=OP.add, op1=OP.mult,
                    )

                    h = h_pool.tile([128, TBLK], f32, tag="h")
                    init = 0.0 if tb == 0 else h_prev[b][hc][:, TBLK - 1:TBLK]
                    nc.vector.tensor_tensor_scan(
                        h[:], a[:], bb[:], init, op0=OP.mult, op1=OP.add
                    )
                    h_prev[b][hc] = h
                    nc.sync.dma_start(out=out_ext[b, hc, :, ts], in_=h[:])

    nc.compile()
    return nc


def _prep_inputs(x, motion_mag, Wz, bz, Wh, bh, motion_weight, motion_bias, alpha):
    x = np.ascontiguousarray(np.asarray(x, dtype=np.float32))
    mm = np.asarray(motion_mag, dtype=np.float32)
    Wz = np.asarray(Wz, dtype=np.float32)
    Wh = np.asarray(Wh, dtype=np.float32)
    bz = np.asarray(bz, dtype=np.float32).reshape(HC, 128, 1)
    bh = np.asarray(bh, dtype=np.float32).reshape(HC, 128, 1)
    mw = float(np.asarray(motion_weight))
    mb = float(np.asarray(motion_bias))
    al = float(np.asarray(alpha))

    a_sp = float(np.log1p(np.exp(al)))  # softplus(alpha)
    sig = 1.0 / (1.0 + np.exp(-(mw * mm + mb)))
    invtau = (1.0 / (1.0 + a_sp * sig)).astype(np.float32)

    wzt = np.ascontiguousarray(Wz.T).reshape(DC, 128, H)
    wht = np.ascontiguousarray(Wh.T).reshape(DC, 128, H)

    in_maps = []
    for c in range(NCORES):
        xl = x[c * BL:(c + 1) * BL].reshape(BL * T, D)
        xt = np.ascontiguousarray(xl.T).reshape(DC, 128, BT)
        in_maps.append({
            "xt": xt,
            "wzt": wzt,
            "wht": wht,
            "bz": bz,
            "bh": bh,
            "invtau": np.ascontiguousarray(
                invtau[c * BL:(c + 1) * BL]).reshape(BL, 1, T),
        })
    return in_maps


def _assemble(results):
    outs = []
    for c in range(NCORES):
        o = results[c]["out"]  # [BL, HC, 128, T]
        o = np.transpose(o, (0, 3, 1, 2)).reshape(BL, T, H)
        outs.append(o)
    return np.ascontiguousarray(np.concatenate(outs, axis=0))


def _run(inputs, trace=False):
    from concourse.bass_utils import run_bass_kernel_spmd

    if "nc" not in _CACHE:
        _CACHE["nc"] = _build_nc()
    nc = _CACHE["nc"]
    in_maps = _prep_inputs(**inputs)
    res = run_bass_kernel_spmd(nc, in_maps, list(range(NCORES)), trace=trace)
    return _assemble(res.results), res


def kernel(**inputs):
    out, _ = _run(inputs, trace=False)
    return out


# revision 23
# speedup vs baseline: 1.0347x; 1.0347x over previous
"""MinGRU cell kernel for Trainium2 (8 NeuronCores, data-parallel over batch).

Computes, for x:[B,T,D], motion_mag:[B,T]:
    tau = 1 + softplus(alpha) * sigmoid(mw*mm + mb)        (per b,t)
    z   = sigmoid((x @ Wz^T + bz) / tau)                   (B,T,H)
    ht  = x @ Wh^T + bh                                    (B,T,H)
    h_t = (1-z_t)*h_{t-1} + z_t*ht_t   (scan over t, h_0=0)

Strategy:
  - Shard B=32 across 8 cores (4 per core). Weights replicated.
  - On-chip layout: h on partitions, t on the free dim, so the recurrence is
    a HW tensor_tensor_scan per [128h, 512t] tile, carried across t-tiles via
    initial=prev[:, -1:].
  - Projections: lhsT = W^T chunks (stationary), rhs = x^T chunks (moving),
    float32r (full PE rate, near-fp32 accuracy, fp32 PSUM accumulation).
  - tau: 1/tau computed on host, DMA-broadcast across partitions per block;
    folded in via one fused scalar_tensor_tensor: u = (zpre + bz) * invtau.
  - z = sigmoid(u), a = sigmoid(-u) = 1-z on ACT; b = (hpre + bh) * z on DVE.
  - Host pre-transposes x to [d, b*t] per core and un-transposes the output.
"""

import sys

import numpy as np

if "/opt/trn_rl_repo" not in sys.path:
    sys.path.insert(0, "/opt/trn_rl_repo")

B, T, D, H = 32, 2048, 512, 512
NCORES = 8
BL = B // NCORES            # batch per core = 4
TBLK = 1024                 # t-columns per block (2 psum banks)
MMN = 512                   # matmul free-dim (1 psum bank)
NTB = T // TBLK             # 2 t-blocks per sample
DC = D // 128               # 4 contraction chunks
HC = H // 128               # 4 h partition chunks
BT = BL * T                 # 8192 columns per core

_CACHE = {}


def _build_nc():
    import concourse.bass as bass
    import concourse.bacc as bacc
    import concourse.mybir as mybir
    import concourse.tile as tile
    from contextlib import ExitStack

    f32 = mybir.dt.float32
    f32r = mybir.dt.float32r
    AF = mybir.ActivationFunctionType
    OP = mybir.AluOpType

    nc = bacc.Bacc("TRN2", target_bir_lowering=False, debug=False)

    xt_ext = nc.declare_dram_parameter("xt", [DC, 128, BT], f32r, isOutput=False)
    wzt_ext = nc.declare_dram_parameter("wzt", [DC, 128, H], f32r, isOutput=False)
    wht_ext = nc.declare_dram_parameter("wht", [DC, 128, H], f32r, isOutput=False)
    bz_ext = nc.declare_dram_parameter("bz", [HC, 128, 1], f32, isOutput=False)
    bh_ext = nc.declare_dram_parameter("bh", [HC, 128, 1], f32, isOutput=False)
    itau_ext = nc.declare_dram_parameter("invtau", [BL, 1, T], f32, isOutput=False)
    out_ext = nc.declare_dram_parameter("out", [BL, HC, 128, T], f32, isOutput=True)

    with tile.TileContext(nc) as tc, ExitStack() as ctx:
        singles = ctx.enter_context(tc.tile_pool(name="singles", bufs=1))
        x_pool = ctx.enter_context(tc.tile_pool(name="x", bufs=2))
        j_pool = ctx.enter_context(tc.tile_pool(name="j", bufs=3))
        psum = ctx.enter_context(tc.tile_pool(name="psum", bufs=2, space="PSUM"))
        work = ctx.enter_context(tc.tile_pool(name="work", bufs=3))
        ab_pool = ctx.enter_context(tc.tile_pool(name="ab", bufs=3))
        h_pool = ctx.enter_context(tc.tile_pool(name="h", bufs=6))

        # Interleave first-block x chunks with Wz chunks so the first
        # matmul's dependencies land as early as possible.
        xs0, wz_sb, wh_sb = [], [], []
        for dc in range(DC):
            wz = singles.tile([128, H], f32r, tag=f"wz{dc}")
            nc.sync.dma_start(out=wz[:], in_=wzt_ext[dc])
            wz_sb.append(wz)
            xt = x_pool.tile([128, TBLK], f32r, tag=f"x{dc}", name=f"x0_{dc}")
            nc.sync.dma_start(out=xt[:], in_=xt_ext[dc, :, 0:TBLK])
            xs0.append(xt)
        for dc in range(DC):
            wh = singles.tile([128, H], f32r, tag=f"wh{dc}")
            nc.sync.dma_start(out=wh[:], in_=wht_ext[dc])
            wh_sb.append(wh)
        bz_col, bh_col = [], []
        for hc in range(HC):
            bzc = singles.tile([128, 1], f32, tag=f"bz{hc}")
            nc.gpsimd.dma_start(out=bzc[:], in_=bz_ext[hc])
            bz_col.append(bzc)
            bhc = singles.tile([128, 1], f32, tag=f"bh{hc}")
            nc.gpsimd.dma_start(out=bhc[:], in_=bh_ext[hc])
            bh_col.append(bhc)

        h_prev = [[None] * HC for _ in range(BL)]

        for b in range(BL):
            for tb in range(NTB):
                bt0 = b * T + tb * TBLK
                ts = slice(tb * TBLK, (tb + 1) * TBLK)
                if b == 0 and tb == 0:
                    xs = xs0
                else:
                    xs = []
                    for dc in range(DC):
                        xt = x_pool.tile([128, TBLK], f32r, tag=f"x{dc}")
                        nc.sync.dma_start(
                            out=xt[:], in_=xt_ext[dc, :, bt0:bt0 + TBLK]
                        )
                        xs.append(xt)
                # Broadcast 1/tau row across all 128 partitions.
                jt = j_pool.tile([128, TBLK], f32, tag="J")
                iv = itau_ext[b, 0, tb * TBLK:(tb + 1) * TBLK]
                iv_b = bass.AP(
                    tensor=iv.tensor, offset=iv.offset, ap=[[0, 128]] + list(iv.ap)
                )
                nc.gpsimd.dma_start(out=jt[:], in_=iv_b)

                for hc in range(HC):
                    hsl = slice(hc * 128, (hc + 1) * 128)
                    zq = psum.tile([128, TBLK], f32, tag="zq")
                    hq = psum.tile([128, TBLK], f32, tag="hq")
                    for half in range(TBLK // MMN):
                        csl = slice(half * MMN, (half + 1) * MMN)
                        for dc in range(DC):
                            nc.tensor.matmul(
                                zq[:, csl],
                                lhsT=wz_sb[dc][:, hsl],
                                rhs=xs[dc][:, csl],
                                start=(dc == 0),
                                stop=(dc == DC - 1),
                            )
                        for dc in range(DC):
                            nc.tensor.matmul(
                                hq[:, csl],
                                lhsT=wh_sb[dc][:, hsl],
                                rhs=xs[dc][:, csl],
                                start=(dc == 0),
                                stop=(dc == DC - 1),
                            )

                    # u = (zpre + bz) * invtau
                    u = work.tile([128, TBLK], f32, tag="u")
                    nc.vector.scalar_tensor_tensor(
                        u[:], zq[:], bz_col[hc][:], jt[:, :],
                        op0=OP.add, op1=OP.mult,
                    )
                    z = work.tile([128, TBLK], f32, tag="z")
                    nc.scalar.activation(z[:], u[:], AF.Sigmoid)
                    a = ab_pool.tile([128, TBLK], f32, tag="a")
                    nc.scalar.activation(a[:], u[:], AF.Sigmoid, scale=-1.0)
                    # b = (hpre + bh) * z
                    bb = ab_pool.tile([128, TBLK], f32, tag="b")
                    nc.vector.scalar_tensor_tensor(
                        bb[:], hq[:], bh_col[hc][:], z[:],
                        op0=OP.add, op1=OP.mult,
                    )

                    h = h_pool.tile([128, TBLK], f32, tag="h")
                    init = 0.0 if tb == 0 else h_prev[b][hc][:, TBLK - 1:TBLK]
                    nc.vector.tensor_tensor_scan(
                        h[:], a[:], bb[:], init, op0=OP.mult, op1=OP.add
                    )
                    h_prev[b][hc] = h
                    nc.sync.dma_start(out=out_ext[b, hc, :, ts], in_=h[:])

    nc.compile()
    return nc


def _prep_inputs(x, motion_mag, Wz, bz, Wh, bh, motion_weight, motion_bias, alpha):
    x = np.ascontiguousarray(np.asarray(x, dtype=np.float32))
    mm = np.asarray(motion_mag, dtype=np.float32)
    Wz = np.asarray(Wz, dtype=np.float32)
    Wh = np.asarray(Wh, dtype=np.float32)
    bz = np.asarray(bz, dtype=np.float32).reshape(HC, 128, 1)
    bh = np.asarray(bh, dtype=np.float32).reshape(HC, 128, 1)
    mw = float(np.asarray(motion_weight))
    mb = float(np.asarray(motion_bias))
    al = float(np.asarray(alpha))

    a_sp = float(np.log1p(np.exp(al)))  # softplus(alpha)
    sig = 1.0 / (1.0 + np.exp(-(mw * mm + mb)))
    invtau = (1.0 / (1.0 + a_sp * sig)).astype(np.float32)

    wzt = np.ascontiguousarray(Wz.T).reshape(DC, 128, H)
    wht = np.ascontiguousarray(Wh.T).reshape(DC, 128, H)

    in_maps = []
    for c in range(NCORES):
        xl = x[c * BL:(c + 1) * BL].reshape(BL * T, D)
        xt = np.ascontiguousarray(xl.T).reshape(DC, 128, BT)
        in_maps.append({
            "xt": xt,
            "wzt": wzt,
            "wht": wht,
            "bz": bz,
            "bh": bh,
            "invtau": np.ascontiguousarray(
                invtau[c * BL:(c + 1) * BL]).reshape(BL, 1, T),
        })
    return in_maps


def _assemble(results):
    outs = []
    for c in range(NCORES):
        o = results[c]["out"]  # [BL, HC, 128, T]
        o = np.transpose(o, (0, 3, 1, 2)).reshape(BL, T, H)
        outs.append(o)
    return np.ascontiguousarray(np.concatenate(outs, axis=0))


def _run(inputs, trace=False):
    from concourse.bass_utils import run_bass_kernel_spmd

    if "nc" not in _CACHE:
        _CACHE["nc"] = _build_nc()
    nc = _CACHE["nc"]
    in_maps = _prep_inputs(**inputs)
    res = run_bass_kernel_spmd(nc, in_maps, list(range(NCORES)), trace=trace)
    return _assemble(res.results), res


def kernel(**inputs):
    out, _ = _run(inputs, trace=False)
    return out
